# revision 70
# baseline (speedup 1.0000x reference)
"""DGCNN (nn_DGCNN_11106785427638) Trainium2 Bass kernel.

Pure data-parallel: B=8 samples sharded 1-per-core across 8 NeuronCores.
Per core (N=2048 points, k=20, f32 throughout):

  Per EdgeConv block b=1..4 (C=3,64,64,64):
    - knn: dist tile (128 rows, 2048) via PE matmul computing the full
      -|x_n - x_m|^2 with the -|x_m|^2 / -|x_n|^2 terms folded in as two
      extra contraction rows (ones-row trick).
    - top-20 via index-embedded bit-surgery keys: q = |x_n-x_m|^2 >= 0;
      the ACT PSUM->SBUF copy computes (q+2^-8)^2 (Square; bits(p2) ~
      2*log2(q'), so a 12-bit mantissa truncation is a RELATIVE 2^-13
      quantization of q), one DVE STT does
      key = (bits(p2) & ~0x7FF) ^ (0x80000000 | m): column index in the
      low mantissa bits, sign flip so f32 max order = ascending-q order.
      16 per-chunk max8 (128 cols each) + a 3-round merge over the 128
      candidates yield sorted top-24 keys; indices decode with one AND --
      no max_index or full-width match_replace passes. Selection is exact
      up to rel-2^-15 (and per-chunk top-8 truncation, P~1e-4/row).
    - neighbor gather runs on GPSIMD ap_gather over A_b = Wa_b @ x_b
      (the 1x1 conv is linear, so conv(gather(x)) == gather(A)); the center
      term Bv_b = (Wb_b - Wa_b) @ x_b is added per-point afterwards.
    - x_{b+1} = prelu(maxpool_j(gather(A)) * s + (Bv*s + bias)) using the
      monotonicity of the (positive-scale) BN + leaky relu.
  Phase 2: y_b = prelu((A_b[idx] )*s + BvS_b) at full (64, N, 20), conv5 as
  two K=128 pair-stacked matmuls, maxpool_j, conv6, global max/mean pool,
  3-layer MLP head. Weights/BN folds are host-preprocessed.

Flat gather order within a 128-row tile: i = a*320 + j*16 + p with local
row n = 16a + p, neighbor rank j. This makes the idx "wrap" for ap_gather a
single 3-dim DMA (via a DRAM bounce), splits phase-1 across all 8 Q7 cores
by row-halves, and keeps pooled outputs n-contiguous.
"""
import sys

sys.path.insert(0, "/opt/trn_rl_repo")
import numpy as np
import concourse.bacc as bacc
import concourse.tile as tile
from concourse import mybir
from concourse.bass_utils import run_bass_kernel_spmd
from contextlib import ExitStack

FP = mybir.dt.float32
U16 = mybir.dt.uint16
I16 = mybir.dt.int16
I32 = mybir.dt.int32
BF = mybir.dt.bfloat16
AF = mybir.ActivationFunctionType
ALU = mybir.AluOpType
AX = mybir.AxisListType

B, N, K, CLS = 8, 2048, 20, 40
NT = N // 128          # 16 row tiles
NEG = -3.0e38
EPS = 1e-5
CS = [3, 64, 64, 64]   # per-block input channels

_CACHE = {}


def _build():
    nc = bacc.Bacc("TRN2", target_bir_lowering=False, debug=False)

    d = {}
    def din(name, shape):
        d[name] = nc.dram_tensor(name, list(shape), FP, kind="ExternalInput").ap()
        return d[name]

    din("x3", (3, N))
    din("onesrow", (1, N))
    d["isign"] = nc.dram_tensor("isign", [128, N], I32, kind="ExternalInput").ap()
    for b in range(1, 5):
        C = CS[b - 1]
        din(f"waaT{b}", (C, 128))
        din(f"wddT{b}", (C, 128))
        din(f"scdup{b}", (128, 1))
        din(f"bidup{b}", (128, 1))
    d["w5T12"] = nc.dram_tensor("w5T12", [128, 64], BF, kind="ExternalInput").ap()
    d["w5T34"] = nc.dram_tensor("w5T34", [128, 64], BF, kind="ExternalInput").ap()
    din("sc5", (64, 1)); din("bi5", (64, 1))
    din("w6Ta", (64, 128)); din("w6Tb", (64, 128))
    din("sc6a", (128, 1)); din("bi6a", (128, 1))
    din("sc6b", (128, 1)); din("bi6b", (128, 1))
    din("l1T", (128, 1024))
    din("sc1a", (128, 1)); din("bi1a", (128, 1))
    din("sc1b", (128, 1)); din("bi1b", (128, 1))
    din("l2T", (128, 128)); din("sc2", (64, 1)); din("bi2", (64, 1))
    din("l3T", (64, CLS)); din("bi3", (CLS, 1))
    out_d = nc.dram_tensor("out", [CLS, 1], FP, kind="ExternalOutput").ap()

    idx_m = {}
    idx_q = {}
    for b in range(1, 5):
        for t in range(NT):
            idx_m[(b, t)] = nc.dram_tensor(f"idxm{b}_{t}", [128, 20], U16)
            idx_q[(b, t)] = nc.dram_tensor(f"idxq{b}_{t}", [16, 160], U16)

    with tile.TileContext(nc) as tc, ExitStack() as ctx:
        persist = ctx.enter_context(tc.tile_pool(name="persist", bufs=1))
        S1 = [persist.tile([CS[b - 1] + 2, N], FP, name=f"S1_{b}", tag=f"S1_{b}") for b in range(1, 5)]
        pairA = {p: persist.tile([128, N], FP, name=f"pairA{p}", tag=f"pairA{p}") for p in (12, 34)}
        BvSpair = {p: persist.tile([128, N], FP, name=f"BvSpair{p}", tag=f"BvSpair{p}") for p in (12, 34)}
        idxw = {p: persist.tile([128, NT * 160], U16, name=f"idxw{p}", tag=f"idxw{p}") for p in (12, 34)}
        idx24_all = [persist.tile([128, NT * 24], U16, name=f"idx24_{i}", tag=f"idx24_{i}") for i in range(4)]
        hmax = persist.tile([64, N], FP)

        # x3 -> S1[0] first: it heads the block-1 critical chain
        nc.sync.dma_start(S1[0][0:3, :], d["x3"][:])

        cp = ctx.enter_context(tc.tile_pool(name="consts", bufs=1))
        sb = {}
        # block-1-critical inputs first so the first dist tile starts ASAP;
        # phase-2/tail weights load AFTER phase 1 is emitted so their DMA
        # dispatch doesn't queue ahead of the per-tile bounce DMAs.
        late = ["w5T12", "w5T34", "sc5", "bi5", "w6Ta", "w6Tb", "sc6a", "bi6a",
                "sc6b", "bi6b", "l1T", "sc1a", "bi1a", "sc1b", "bi1b",
                "l2T", "sc2", "bi2", "l3T", "bi3"]
        mid = [f"{pfx}{b_}" for b_ in range(2, 5)
               for pfx in ("waaT", "wddT", "scdup", "bidup")]
        early = ["isign", "waaT1", "wddT1", "scdup1", "bidup1"]
        for name in early + [n for n in d if n not in early]:
            if name in ("x3", "onesrow"):
                continue  # read directly from DRAM; no SBUF copy needed
            ap = d[name]
            t_ = cp.tile(list(ap.shape), ap.dtype, name=f"c_{name}", tag=f"c_{name}")
            if name not in late and name not in mid:
                nc.sync.dma_start(t_[:], ap[:])
            sb[name] = t_
        posones = cp.tile([64, 1], FP)
        nc.vector.memset(posones[:], 1.0)
        isign = sb["isign"]
        c_mask = cp.tile([128, 1], I32)
        nc.vector.memset(c_mask[:], -2048)     # 0xFFFFF800
        c_2047 = cp.tile([128, 1], I32)
        nc.vector.memset(c_2047[:], 2047)
        c_eps = cp.tile([128, 1], FP)
        nc.vector.memset(c_eps[:], 2.0 ** -8)


        # ---------------- phase 1: blocks ----------------
        with tc.tile_pool(name="pre", bufs=2) as prep, \
             tc.tile_pool(name="xsqp", bufs=1) as xsqp, \
             tc.tile_pool(name="adup", bufs=2) as adupp, \
             tc.tile_pool(name="bvs", bufs=2) as bvsp, \
             tc.tile_pool(name="dist", bufs=2) as distp, \
             tc.tile_pool(name="gph1", bufs=4) as gph1p, \
             tc.tile_pool(name="small", bufs=4) as smallp, \
             tc.tile_pool(name="idxq", bufs=3) as idxqp, \
             tc.tile_pool(name="ps_pre", bufs=1, space="PSUM") as ps_pre, \
             tc.tile_pool(name="ps_dist", bufs=2, space="PSUM") as ps_dist:
            S2s, xsqs, Adups, BvSds = {}, {}, {}, {}

            def prestep_core(b, c0, cn):
                # S2-critical prestep for block b, columns [c0, c0+cn): S1 rows
                # [x; 1; xx], S2 rows [-2x; xx; 1] so the dist matmul yields
                # q[n,m] = xx_n - 2<x_n,x_m> + xx_m >= 0. Emitted piecewise
                # inside block b-1's tile loop to overlap the transition.
                C = CS[b - 1]
                x_b = S1[b - 1][0:C, :]
                hs = slice(c0, c0 + cn)
                if c0 == 0:
                    xsqs[b] = xsqp.tile([C, N], FP, tag="xsq", name=f"xsq{b}")
                    S2s[b] = prep.tile([C + 2, N], FP, tag="S2", name=f"S2_{b}")
                    if b <= 3:
                        Adups[b] = adupp.tile([128, N], FP, tag="adup", name=f"Adup{b}")
                        BvSds[b] = bvsp.tile([128, N], FP, tag="bvs", name=f"BvSd{b}")
                xsq, S2 = xsqs[b], S2s[b]
                nc.scalar.activation(xsq[:, hs], x_b[:, hs], AF.Square)
                pxx = ps_pre.tile([1, cn], FP, tag="pre1")
                for ch in range(cn // 512):
                    ci = slice(c0 + ch * 512, c0 + (ch + 1) * 512)
                    nc.tensor.matmul(pxx[:, ch * 512:(ch + 1) * 512], posones[0:C, :],
                                     xsq[:, ci], start=True, stop=True)
                if c0 == 0:
                    # the constant rows (ones) for ALL columns up front: their
                    # only dep is the S2 buffer's WAR (long since clear), so
                    # they leave the later pieces' critical chains.
                    nc.sync.dma_start(S1[b - 1][C:C + 1, :], d["onesrow"][:])
                    nc.sync.dma_start(S2[C + 1:C + 2, :], d["onesrow"][:])
                nc.scalar.mul(S2[0:C, hs], x_b[:, hs], -2.0)
                if C == 64:
                    # base-64 is 32-aligned: ACT can write the xx row directly
                    nc.scalar.copy(S2[C:C + 1, hs], pxx[:])
                    nc.sync.dma_start(S1[b - 1][C + 1:C + 2, hs], S2[C:C + 1, hs])
                else:
                    nc.scalar.copy(xsq[0:1, hs], pxx[:])
                    nc.sync.dma_start(S2[C:C + 1, hs], xsq[0:1, hs])
                    nc.sync.dma_start(S1[b - 1][C + 1:C + 2, hs], xsq[0:1, hs])

            def prestep_ab(b, h):
                # pairA / Adup / BvS products for block b, half h (off the
                # dist critical chain; gathers need them a few us later)
                C = CS[b - 1]
                pair = 12 if b <= 2 else 34
                half = slice(0, 64) if b % 2 == 1 else slice(64, 128)
                x_b = S1[b - 1][0:C, :]
                hs = slice(h * 1024, (h + 1) * 1024)
                for wname in ("waaT", "wddT"):
                    ab = ps_pre.tile([128, 1024], FP, tag="pre2")
                    for ch in range(2):
                        ci = slice(h * 1024 + ch * 512, h * 1024 + (ch + 1) * 512)
                        nc.tensor.matmul(ab[:, ch * 512:(ch + 1) * 512],
                                         sb[f"{wname}{b}"][:], x_b[:, ci],
                                         start=True, stop=True)
                    if wname == "waaT":
                        nc.scalar.copy(pairA[pair][half, hs], ab[half, :])
                        if b <= 3:
                            nc.scalar.copy(Adups[b][:, hs], ab[:])
                    else:
                        nc.scalar.activation(BvSpair[pair][half, hs], ab[half, :],
                                             AF.Identity,
                                             bias=sb[f"bidup{b}"][half, 0:1],
                                             scale=sb[f"scdup{b}"][half, 0:1])
                        if b <= 3:
                            nc.scalar.activation(BvSds[b][:, hs], ab[:], AF.Identity,
                                                 bias=sb[f"bidup{b}"][:, 0:1],
                                                 scale=sb[f"scdup{b}"][:, 0:1])

            prestep_core(1, 0, 1024)
            prestep_core(1, 1024, 1024)
            prestep_ab(1, 0)
            prestep_ab(1, 1)
            # non-block-1 weights: emitted after block 1's prestep so their
            # DMA dispatch doesn't delay the first dist tiles
            for b_ in range(2, 5):
                for pfx in ("waaT", "wddT", "scdup", "bidup"):
                    nm = f"{pfx}{b_}"
                    nc.sync.dma_start(sb[nm][:], d[nm][:])
            def pool_path(b, t, G):
                # maxpool over the 20 gathered neighbors -> BvS add -> prelu
                # -> x_{b+1} tile write
                BvSd = BvSds[b]
                Rt = smallp.tile([128, 64], FP, tag="rt")
                nc.vector.tensor_reduce(
                    Rt[:], G[:].rearrange("c (a j p) -> c a p j", a=4, j=20, p=16),
                    AX.X, ALU.max)
                t1 = smallp.tile([128, 64], FP, tag="t1")
                nc.vector.tensor_tensor(
                    t1[0:64, :], Rt[0:64, :],
                    BvSd[0:64, t * 128:t * 128 + 64], ALU.add)
                nc.vector.tensor_tensor(
                    t1[64:128, :], Rt[64:128, :],
                    BvSd[64:128, t * 128 + 64:(t + 1) * 128], ALU.add)
                t2 = smallp.tile([128, 64], FP, tag="t2")
                nc.scalar.activation(t2[:], t1[:], AF.Prelu, alpha=0.2)
                nc.scalar.copy(S1[b][0:64, t * 128:t * 128 + 64], t2[0:64, :])
                nc.scalar.dma_start(S1[b][0:64, t * 128 + 64:(t + 1) * 128],
                                    t2[64:128, :])

            for b in range(1, 5):
                C = CS[b - 1]
                pair = 12 if b <= 2 else 34
                half = slice(0, 64) if b % 2 == 1 else slice(64, 128)
                qbase = 0 if b % 2 == 1 else 4
                S2 = S2s[b]
                Adup = Adups.get(b)
                pend = []

                for t in range(NT):
                    lhsT = S1[b - 1][:, t * 128:(t + 1) * 128]
                    key = distp.tile([128, N], FP, tag="dist")
                    for ch in range(2):
                        dps = ps_dist.tile([128, 1024], FP, tag="dch")
                        for u in range(2):
                            nc.tensor.matmul(dps[:, u * 512:(u + 1) * 512], lhsT,
                                             S2[:, ch * 1024 + u * 512:ch * 1024 + (u + 1) * 512],
                                             start=True, stop=True)
                        nc.scalar.activation(key[:, ch * 1024:(ch + 1) * 1024], dps[:],
                                             AF.Square, bias=c_eps[:, 0:1], scale=1.0)
                    # p4 = (q + 2^-8)^4: 12-bit mantissa trunc => rel-2^-14 on q
                    # key = (bits(p2) & ~0x7FF) ^ (0x80000000 | m)
                    nc.vector.scalar_tensor_tensor(key[:].bitcast(I32), key[:].bitcast(I32),
                                                   c_mask[:], isign[:],
                                                   op0=ALU.bitwise_and, op1=ALU.bitwise_xor)
                    allc = smallp.tile([128, 128], FP, tag="allc")
                    for c in range(16):
                        nc.vector.max(allc[:, c * 8:(c + 1) * 8],
                                      key[:, c * 128:(c + 1) * 128])
                    key24 = smallp.tile([128, 24], FP, tag="key24")
                    for r in range(3):
                        nc.vector.max(key24[:, r * 8:(r + 1) * 8], allc[:])
                        if r < 2:
                            nc.vector.match_replace(allc[:], key24[:, r * 8:(r + 1) * 8],
                                                    allc[:], NEG)
                    # decode: idx = bits(key24) & 0x7FF
                    i24 = smallp.tile([128, 24], I32, tag="i24")
                    idx24 = idx24_all[b - 1][:, t * 24:(t + 1) * 24]
                    nc.vector.tensor_scalar(i24[:], key24[:].bitcast(I32), c_2047[:], None,
                                            op0=ALU.bitwise_and)
                    nc.vector.tensor_copy(idx24[:], i24[:])

                    # idx wrap via double DRAM bounce: m (natural) -> q0 (wrapped)
                    # -> Q (wrapped DRAM); idxw/idxh then replicate across
                    # 16-partition groups with single broadcast-src DMAs.
                    m_ap = idx_m[(b, t)].ap()
                    nc.sync.dma_start(m_ap, idx24[:, 0:20])
                    q0 = idxqp.tile([16, 160], U16, tag="q0")
                    nc.sync.dma_start(q0[:], m_ap.rearrange("(a p) j -> p a j", a=8, p=16))
                    q_ap = idx_q[(b, t)].ap()
                    nc.sync.dma_start(q_ap, q0[:])
                    nc.sync.dma_start(
                        idxw[pair][qbase * 16:(qbase + 4) * 16, t * 160:(t + 1) * 160],
                        q_ap.rearrange("p (u c) -> u p c", u=1).broadcast_to([4, 16, 160]))

                    if t == 0 and b >= 2:
                        # must precede tile-0's gather: it reads Adup half 1
                        prestep_ab(b, 1)

                    if b <= 3:
                        idxh = smallp.tile([128, 80], U16, tag="idxh")
                        if t >= NT - 3:
                            # last tiles sit on the block-transition drain:
                            # skip the q0->Q hops with direct (replicated)
                            # reads of the natural-layout bounce
                            for h in range(2):
                                msrc = m_ap[h * 64:(h + 1) * 64, :].rearrange(
                                    "(a p) j -> p a j", a=4, p=16)
                                for u in range(4):
                                    nc.scalar.dma_start(
                                        idxh[(h * 4 + u) * 16:(h * 4 + u + 1) * 16, :]
                                        .rearrange("p (a j) -> p a j", a=4),
                                        msrc)
                        else:
                            for h in range(2):
                                nc.scalar.dma_start(
                                    idxh[h * 64:(h + 1) * 64, :],
                                    q_ap[:, h * 80:(h + 1) * 80]
                                    .rearrange("p (u c) -> u p c", u=1)
                                    .broadcast_to([4, 16, 80]))
                        G = gph1p.tile([128, 1280], FP, tag="g1")
                        nc.gpsimd.ap_gather(G[:], Adup[:], idxh[:].bitcast(I16),
                                            channels=128, num_elems=N, d=1, num_idxs=1280)
                        # pool path deferred two tiles: absorbs the
                        # bounce-DMA + gather latency so the DVE never
                        # head-of-line stalls on its own gather.
                        if len(pend) == 3:
                            pool_path(b, *pend.pop(0))
                        pend.append((t, G))

                    if b < 4:
                        if t == 10:
                            prestep_core(b + 1, 0, 1024)
                        elif t == 12:
                            prestep_ab(b + 1, 0)
                        elif t == 14:
                            prestep_core(b + 1, 1024, 512)
                while pend:
                    pool_path(b, *pend.pop(0))
                if b < 4:
                    prestep_core(b + 1, 1536, 512)

        # late consts: phase-2 / tail weights stream in during phase 1's tail
        for name in late:
            nc.sync.dma_start(sb[name][:], d[name][:])

        # ---------------- phase 2: y + conv5 + pool ----------------
        # y = prelu(gather(A) + BvS_bcast); the prelu ACT pass downcasts y to
        # bf16 so the conv5 matmuls run at the PE's bf16 rate (output error
        # ~1e-4: y feeds conv+maxpool only).
        with tc.tile_pool(name="g2", bufs=2) as g2p, \
             tc.tile_pool(name="y2", bufs=2) as y2p, \
             tc.tile_pool(name="hsb", bufs=2) as hsbp, \
             tc.tile_pool(name="t6", bufs=2) as t6p, \
             tc.tile_pool(name="ps_h", bufs=3, space="PSUM") as ps_h, \
             tc.tile_pool(name="ps6", bufs=2, space="PSUM") as ps6:
            def phase2_front(t):
                ys = {}
                for pair in (12, 34):
                    G = g2p.tile([128, 2560], FP, name=f"g{pair}_{t}", tag=f"g{pair}")
                    nc.gpsimd.ap_gather(G[:], pairA[pair][:],
                                        idxw[pair][:, t * 160:(t + 1) * 160].bitcast(I16),
                                        channels=128, num_elems=N, d=1, num_idxs=2560)
                    y = y2p.tile([128, 2560], BF, name=f"y{pair}_{t}", tag=f"y{pair}")
                    for a in range(8):
                        gv = G[:, a * 320:(a + 1) * 320].rearrange(
                            "c (j p) -> c j p", j=20, p=16)
                        bvv = BvSpair[pair][:, t * 128 + a * 16:t * 128 + (a + 1) * 16] \
                            .rearrange("c (u2 p) -> c u2 p", u2=1) \
                            .broadcast_to([128, 20, 16])
                        nc.vector.scalar_tensor_tensor(gv, gv, 1.0, bvv,
                                                       op0=ALU.mult, op1=ALU.add)
                    for g in range(4):
                        nc.scalar.activation(y[:, g * 640:(g + 1) * 640],
                                             G[:, g * 640:(g + 1) * 640],
                                             AF.Prelu, alpha=0.2)
                    ys[pair] = y
                return ys

            def phase2_back(t, ys):
                h_sb = hsbp.tile([64, 2560], FP, name=f"hsb_{t}", tag="hsb")
                for ch in range(5):
                    cs = slice(ch * 512, (ch + 1) * 512)
                    hps = ps_h.tile([64, 512], FP, name=f"hps_{t}_{ch}", tag="h")
                    nc.tensor.matmul(hps[:], sb["w5T12"][:], ys[12][:, cs],
                                     start=True, stop=False)
                    nc.tensor.matmul(hps[:], sb["w5T34"][:], ys[34][:, cs],
                                     start=False, stop=True)
                    nc.scalar.activation(h_sb[:, cs], hps[:], AF.Prelu,
                                         bias=sb["bi5"][:, 0:1], scale=sb["sc5"][:, 0:1],
                                         alpha=0.2)
                nc.vector.tensor_reduce(
                    hmax[:, t * 128:(t + 1) * 128],
                    h_sb[:].rearrange("c (a j p) -> c a p j", a=8, j=20, p=16),
                    AX.X, ALU.max)

            # conv6 interleaved per 512-col group: z6 chunk matmul as soon as
            # its 4 hmax tiles land; prelu with accum_out gives the sum-pool
            # for free, one small DVE reduce gives the max-pool partial.
            gparts = {}
            for wname in ("w6Ta", "w6Tb"):
                gparts[wname] = (cp.tile([128, 4], FP, name=f"gmp_{wname}"),
                                 cp.tile([128, 4], FP, name=f"gsp_{wname}"))

            def conv6_chunk(g):
                cs = slice(g * 512, (g + 1) * 512)
                for wname, scn, bin_ in (("w6Ta", "sc6a", "bi6a"),
                                         ("w6Tb", "sc6b", "bi6b")):
                    z6 = ps6.tile([128, 512], FP, tag="z6")
                    nc.tensor.matmul(z6[:], sb[wname][:], hmax[:, cs],
                                     start=True, stop=True)
                    h6c = t6p.tile([128, 512], FP, tag="h6c")
                    gmp, gsp = gparts[wname]
                    nc.scalar.activation(h6c[:], z6[:], AF.Prelu,
                                         bias=sb[bin_][:, 0:1], scale=sb[scn][:, 0:1],
                                         alpha=0.2, accum_out=gsp[:, g:g + 1])
                    nc.vector.tensor_reduce(gmp[:, g:g + 1], h6c[:], AX.X, ALU.max)

            ys_prev = None
            for t in range(NT + 1):
                ys_cur = phase2_front(t) if t < NT else None
                if ys_prev is not None:
                    phase2_back(t - 1, ys_prev)
                    if (t - 1) % 4 == 3:
                        conv6_chunk((t - 1) // 4)
                ys_prev = ys_cur

        # ---------------- tail: global pool combine, MLP ----------------
        with tc.tile_pool(name="tail", bufs=1) as tp, \
             tc.tile_pool(name="ps_fc", bufs=2, space="PSUM") as ps_fc:
            gpieces = []
            for wname in ("w6Ta", "w6Tb"):
                gmp, gsp = gparts[wname]
                gm = tp.tile([128, 1], FP, tag=f"gm{wname}")
                nc.vector.tensor_reduce(gm[:], gmp[:], AX.X, ALU.max)
                gs = tp.tile([128, 1], FP, tag=f"gs{wname}")
                nc.vector.tensor_reduce(gs[:], gsp[:], AX.X, ALU.add)
                gpieces.append((gm, gs))
            gchunks = [gpieces[0][0], gpieces[1][0], gpieces[0][1], gpieces[1][1]]

            z1sb = tp.tile([128, 2], FP, tag="z1")
            for o in range(2):
                z1 = ps_fc.tile([128, 1], FP, tag="fc")
                for k in range(4):
                    nc.tensor.matmul(z1[:], sb["l1T"][:, (k * 2 + o) * 128:(k * 2 + o + 1) * 128],
                                     gchunks[k][:], start=(k == 0), stop=(k == 3))
                nc.scalar.activation(z1sb[:, o:o + 1], z1[:], AF.Prelu,
                                     bias=sb["bi1a" if o == 0 else "bi1b"][:, 0:1],
                                     scale=sb["sc1a" if o == 0 else "sc1b"][:, 0:1],
                                     alpha=0.01)
            z2 = ps_fc.tile([64, 1], FP, tag="fc")
            nc.tensor.matmul(z2[:], sb["l2T"][:, 0:64], z1sb[:, 0:1], start=True, stop=False)
            nc.tensor.matmul(z2[:], sb["l2T"][:, 64:128], z1sb[:, 1:2], start=False, stop=True)
            z2sb = tp.tile([64, 1], FP, tag="z2")
            nc.scalar.activation(z2sb[:], z2[:], AF.Prelu,
                                 bias=sb["bi2"][:, 0:1], scale=sb["sc2"][:, 0:1],
                                 alpha=0.01)
            z3 = ps_fc.tile([CLS, 1], FP, tag="fc")
            nc.tensor.matmul(z3[:], sb["l3T"][:], z2sb[:], start=True, stop=True)
            z3sb = tp.tile([CLS, 1], FP, tag="z3")
            nc.scalar.activation(z3sb[:], z3[:], AF.Identity, bias=sb["bi3"][:, 0:1])
            nc.sync.dma_start(out_d[:], z3sb[:])

    nc.compile()
    return nc


def _host_prep(inputs):
    f32 = np.float32

    def bnfold(p):
        s, b, m, v = np.asarray(p, f32)
        scl = (s / np.sqrt(v + EPS)).astype(f32)
        return scl, (b - m * scl).astype(f32)

    w = {}
    for b in range(1, 5):
        C = CS[b - 1]
        wb = np.asarray(inputs[f"w{b}"], f32)
        wa, wrest = wb[:, :C], wb[:, C:]
        wd = (wrest - wa).astype(f32)
        scl, bi = bnfold(inputs[f"bn{b}"])
        w[f"waaT{b}"] = (np.concatenate([wa.T, wa.T], axis=1) * np.tile(scl, 2)[None, :]).astype(f32)
        w[f"wddT{b}"] = np.concatenate([wd.T, wd.T], axis=1).astype(f32)
        w[f"scdup{b}"] = np.tile(scl, 2)[:, None]
        w[f"bidup{b}"] = np.tile(bi, 2)[:, None]
        w.setdefault("_scl", {})[b] = (scl, bi)
    scl1, bi1 = w["_scl"][1]; scl2, bi2 = w["_scl"][2]
    scl3, bi3 = w["_scl"][3]; scl4, bi4 = w["_scl"][4]
    del w["_scl"]

    w5 = np.asarray(inputs["w5"], f32)
    import ml_dtypes
    w["w5T12"] = np.ascontiguousarray(w5[:, 0:128].T).astype(ml_dtypes.bfloat16)
    w["w5T34"] = np.ascontiguousarray(w5[:, 128:256].T).astype(ml_dtypes.bfloat16)
    s5, b5 = bnfold(inputs["bn5"])
    w["sc5"], w["bi5"] = s5[:, None], b5[:, None]

    w6 = np.asarray(inputs["w6"], f32)
    w["w6Ta"] = w6[0:128, :].T.copy()
    w["w6Tb"] = w6[128:256, :].T.copy()
    s6, b6 = bnfold(inputs["bn6"])
    w["sc6a"], w["bi6a"] = s6[0:128, None], b6[0:128, None]
    w["sc6b"], w["bi6b"] = s6[128:256, None], b6[128:256, None]

    lw1 = np.asarray(inputs["lw1"], f32)
    lb1 = np.asarray(inputs["lb1"], f32)
    sl1, bb1 = bnfold(inputs["lbn1"])
    bias1 = (lb1 * sl1 + bb1).astype(f32)
    LW1 = np.concatenate([lw1[:, 0:256], lw1[:, 256:512] / 2048.0], axis=1).astype(f32)
    l1T = np.zeros((128, 1024), f32)
    for k in range(4):
        for o in range(2):
            l1T[:, (k * 2 + o) * 128:(k * 2 + o + 1) * 128] = \
                LW1[o * 128:(o + 1) * 128, k * 128:(k + 1) * 128].T
    w["l1T"] = l1T
    w["sc1a"], w["bi1a"] = sl1[0:128, None], bias1[0:128, None]
    w["sc1b"], w["bi1b"] = sl1[128:256, None], bias1[128:256, None]

    lw2 = np.asarray(inputs["lw2"], f32)
    lb2 = np.asarray(inputs["lb2"], f32)
    sl2, bb2 = bnfold(inputs["lbn2"])
    l2T = np.zeros((128, 128), f32)
    l2T[:, 0:64] = lw2[:, 0:128].T
    l2T[:, 64:128] = lw2[:, 128:256].T
    w["l2T"] = l2T
    w["sc2"] = sl2[:, None]
    w["bi2"] = (lb2 * sl2 + bb2)[:, None]

    w["l3T"] = np.asarray(inputs["lw3"], f32).T.copy()
    w["bi3"] = np.asarray(inputs["lb3"], f32)[:, None]
    return w


def kernel(**inputs):
    if "nc" not in _CACHE:
        _CACHE["nc"] = _build()
    nc = _CACHE["nc"]
    w = _host_prep(inputs)
    x = np.asarray(inputs["x"], np.float32)
    isign = np.tile(((np.arange(N, dtype=np.uint32) | np.uint32(0x80000000))
                     .view(np.int32))[None, :], (128, 1))
    in_maps = []
    for i in range(B):
        m = dict(w)
        m["x3"] = np.ascontiguousarray(x[i])
        m["onesrow"] = np.ones((1, N), np.float32)
        m["isign"] = isign
        in_maps.append(m)
    res = run_bass_kernel_spmd(nc, in_maps, list(range(B)))
    return np.stack([res.results[i]["out"].reshape(CLS) for i in range(B)]).astype(np.float32)

